# revision 1
# baseline (speedup 1.0000x reference)
"""HGNN layer kernel for 8 Trainium2 NeuronCores (v2: dma_gather + bf16 hi/lo).

Reference:
    X_norm = X * DV_inv_sqrt[:, None]
    HX     = segment_sum(X_norm[h_rows] * h_vals[:,None], h_cols, E) * DE_inv[:,None]
    X_out  = segment_sum(HX[h_cols] * h_vals[:,None], h_rows, N) * DV_inv_sqrt[:,None]
    return X_out @ W.T + b

Strategy (requires h_vals == 1, which the problem guarantees; otherwise a
numpy fallback runs): all normalization folds into host-precomputed tables,
so the device-side scatter matrix is an exact 0/1 one-hot that can be bf16.
Tables are stored as interleaved bf16 (hi | lo) rows, hi = bf16(x),
lo = bf16(x - hi), so one 512B dma_gather row carries an exact fp32-grade
pair; each chunk then does two bf16 matmuls accumulating into fp32 PSUM.

Pass 1 (edges sharded, 3125/core): windows of 128 edges; entries of a window
split by node half (int16 index limit), bulk-gathered by two dma_gathers
from the two half tables; per 128-entry chunk S = (iota == col_local) bf16,
PSUM[wsz,128] += S^T @ G_hi + S^T @ G_lo.
Host: HX_norm = HX * DE_inv -> hi/lo table.
Pass 2 (nodes sharded, 6250/core): same against HX table (single gather),
accumulated transposed [D, wsz], then the Linear as lhsT = W^T (bf16 hi/lo
of W applied as two matmuls against the fp32->bf16 hi/lo of the window
result would cost extra; instead W matmul runs on the fp32 window result
copied to SBUF in bf16 hi/lo pair) -> OUT^T [128, 6250] per core; host
applies DV_inv_sqrt scaling and bias (they commute through the Linear).
"""

import numpy as np
import ml_dtypes

import concourse.bacc as bacc
import concourse.bass as bass
import concourse.mybir as mybir
import concourse.tile as tile
from concourse.bass_utils import run_bass_kernel_spmd

N, E, NNZ, D = 50000, 25000, 600000, 128
C = 8
EPC = E // C
NPC = N // C
P = 128
HALF = 25000  # pass-1 node-table split point (int16 index limit)
F32 = mybir.dt.float32
BF16 = mybir.dt.bfloat16
I16 = mybir.dt.int16

TRACE = False
LAST_EXEC_NS = []
LAST_RESULTS = []


def _hi_lo_table(x):
    """[R, D] f32 -> [R, 2*D] bf16 interleaved row: [hi | lo]."""
    hi = x.astype(ml_dtypes.bfloat16)
    lo = (x - hi.astype(np.float32)).astype(ml_dtypes.bfloat16)
    return np.ascontiguousarray(np.concatenate([hi, lo], axis=1))


def _pack(loc_all, idx_all, rows_out, split_at):
    """Pack per-core entries (sorted by local out-row) into window groups.

    Returns (idx16 [C,128,TCI], loc [C,128,TCC] bf16, ncw_a, ncw_b,
    n_windows, win_sizes). Window w occupies chunk cols
    [w*(ncw_a+ncw_b), ...) with half-A chunks first; idx cols likewise in
    16-wrapped units of 8 per chunk. Pad slots: idx=0, loc=255.
    """
    n_windows = (rows_out + P - 1) // P
    win_sizes = [min(P, rows_out - w * P) for w in range(n_windows)]
    per_core = []
    ncw_a = ncw_b = 1
    for c in range(C):
        loc = loc_all[c]
        idx = idx_all[c]
        order = np.argsort(loc, kind="stable")
        locs, idxs = loc[order], idx[order]
        win = locs // P
        starts = np.searchsorted(win, np.arange(n_windows))
        ends = np.searchsorted(win, np.arange(n_windows) + 1)
        wins = []
        for w in range(n_windows):
            lw, iw = locs[starts[w] : ends[w]], idxs[starts[w] : ends[w]]
            if split_at is not None:
                ma = iw < split_at
                la, ia = lw[ma], iw[ma]
                lb, ib = lw[~ma], iw[~ma] - split_at
            else:
                la, ia = lw, iw
                lb = ib = np.zeros(0, np.int64)
            wins.append((la, ia, lb, ib))
            ncw_a = max(ncw_a, -(-len(la) // P))
            ncw_b = max(ncw_b, -(-len(lb) // P)) if split_at is not None else 0
        per_core.append(wins)
    if split_at is None:
        ncw_b = 0
    cw = ncw_a + ncw_b
    tcc = n_windows * cw
    idx16 = np.zeros((C, 16, tcc * 8), np.int16)
    locg = np.full((C, P, tcc), 255.0, dtype=np.float32)
    for c in range(C):
        for w, (la, ia, lb, ib) in enumerate(per_core[c]):
            for half, (lh, ih, ncw, coff) in enumerate(
                [(la, ia, ncw_a, 0), (lb, ib, ncw_b, ncw_a)]
            ):
                if ncw == 0:
                    continue
                base = w * cw + coff
                n = len(lh)
                arr = np.zeros(ncw * P, np.int16)
                arr[:n] = ih
                idx16[c, :, base * 8 : (base + ncw) * 8] = arr.reshape(ncw * 8, 16).T
                k = np.arange(n)
                locg[c, k % P, base + k // P] = (lh - w * P).astype(np.float32)
    idx16 = np.ascontiguousarray(np.tile(idx16, (1, 8, 1)))
    return idx16, locg, ncw_a, ncw_b, n_windows, win_sizes


def _build(ncw_a, ncw_b, n_windows, win_sizes, pass2):
    """Unified builder. pass1: two half tables, out [EPC, D] f32 direct.
    pass2: one table, transposed accum + Linear, out [D, NPC] f32."""
    cw = ncw_a + ncw_b
    tcc = n_windows * cw
    nc = bacc.Bacc("TRN2", target_bir_lowering=False, debug=False, num_devices=C)
    ta = nc.dram_tensor("ta", [HALF, 2 * D], BF16, kind="ExternalInput")
    if not pass2:
        tb = nc.dram_tensor("tb", [N - HALF, 2 * D], BF16, kind="ExternalInput")
    idx_d = nc.dram_tensor("idx", [P, tcc * 8], I16, kind="ExternalInput")
    loc_d = nc.dram_tensor("loc", [P, tcc], F32, kind="ExternalInput")
    iota_d = nc.dram_tensor("iota", [P, P], BF16, kind="ExternalInput")
    if pass2:
        wt_d = nc.dram_tensor("wt", [D, 2 * D], BF16, kind="ExternalInput")
        out_d = nc.dram_tensor("out", [D, NPC], F32, kind="ExternalOutput")
    else:
        out_d = nc.dram_tensor("out", [EPC, D], F32, kind="ExternalOutput")

    with tile.TileContext(nc) as t:
        with (
            t.tile_pool(name="const", bufs=1) as cpool,
            t.tile_pool(name="gath", bufs=3) as gpool,
            t.tile_pool(name="sel", bufs=4) as spool,
            t.tile_pool(name="mid", bufs=2) as mpool,
            t.tile_pool(name="outp", bufs=2) as opool,
            t.tile_pool(name="psum", bufs=2, space="PSUM") as ppool,
            t.tile_pool(name="psum2", bufs=2, space="PSUM") as ppool2,
        ):
            idx_sb = cpool.tile([P, tcc * 8], I16)
            loc_sb = cpool.tile([P, tcc], F32)
            iota_sb = cpool.tile([P, P], BF16)
            nc.sync.dma_start(out=idx_sb[:], in_=idx_d[:])
            nc.sync.dma_start(out=loc_sb[:], in_=loc_d[:])
            nc.sync.dma_start(out=iota_sb[:], in_=iota_d[:])
            if pass2:
                wt_sb = cpool.tile([D, 2 * D], BF16)
                nc.sync.dma_start(out=wt_sb[:], in_=wt_d[:])

            for w in range(n_windows):
                wsz = win_sizes[w]
                base = w * cw
                g = gpool.tile([P, cw, 2 * D], BF16, tag="g")
                nc.gpsimd.dma_gather(
                    g[:, :ncw_a, :],
                    ta[:],
                    idx_sb[:, base * 8 : (base + ncw_a) * 8],
                    ncw_a * P,
                    ncw_a * P,
                    2 * D,
                    single_packet=False,
                )
                if ncw_b:
                    nc.gpsimd.dma_gather(
                        g[:, ncw_a:cw, :],
                        tb[:],
                        idx_sb[:, (base + ncw_a) * 8 : (base + cw) * 8],
                        ncw_b * P,
                        ncw_b * P,
                        2 * D,
                        single_packet=False,
                    )
                ps = ppool.tile([D, wsz] if pass2 else [wsz, D], F32, tag="ps")
                for j in range(cw):
                    s = spool.tile([P, wsz], BF16, tag="s")
                    nc.vector.tensor_scalar(
                        out=s[:],
                        in0=iota_sb[:, :wsz],
                        scalar1=loc_sb[:, base + j : base + j + 1],
                        scalar2=None,
                        op0=mybir.AluOpType.is_equal,
                    )
                    for h in range(2):
                        gj = g[:, j, h * D : (h + 1) * D]
                        if pass2:
                            nc.tensor.matmul(
                                out=ps[:],
                                lhsT=gj,
                                rhs=s[:],
                                start=(j == 0 and h == 0),
                                stop=(j == cw - 1 and h == 1),
                            )
                        else:
                            nc.tensor.matmul(
                                out=ps[:],
                                lhsT=s[:],
                                rhs=gj,
                                start=(j == 0 and h == 0),
                                stop=(j == cw - 1 and h == 1),
                            )
                if pass2:
                    # hi/lo of window result, then Linear: p2 = W @ x
                    # = Whi@xhi + Whi@xlo + Wlo@xhi  (Wlo@xlo ~ 2^-18, drop)
                    thi = mpool.tile([D, wsz], BF16, tag="thi")
                    tlo = mpool.tile([D, wsz], BF16, tag="tlo")
                    nc.vector.tensor_copy(out=thi[:], in_=ps[:])
                    nc.vector.tensor_tensor(
                        out=tlo[:], in0=ps[:], in1=thi[:],
                        op=mybir.AluOpType.subtract,
                    )
                    p2 = ppool2.tile([D, wsz], F32, tag="p2")
                    nc.tensor.matmul(
                        out=p2[:], lhsT=wt_sb[:, :D], rhs=thi[:],
                        start=True, stop=False,
                    )
                    nc.tensor.matmul(
                        out=p2[:], lhsT=wt_sb[:, :D], rhs=tlo[:],
                        start=False, stop=False,
                    )
                    nc.tensor.matmul(
                        out=p2[:], lhsT=wt_sb[:, D:], rhs=thi[:],
                        start=False, stop=True,
                    )
                    o = opool.tile([D, wsz], F32, tag="o")
                    nc.vector.tensor_copy(out=o[:], in_=p2[:])
                    nc.sync.dma_start(
                        out=out_d[:, w * P : w * P + wsz], in_=o[:]
                    )
                else:
                    o = opool.tile([wsz, D], F32, tag="o")
                    nc.vector.tensor_copy(out=o[:], in_=ps[:])
                    nc.sync.dma_start(
                        out=out_d[w * P : w * P + wsz, :], in_=o[:]
                    )
    nc.compile()
    return nc


def _kernel_np(X, rows, cols, vals, dv, de, W, b):
    Xn = X * dv[:, None]
    msg = Xn[rows] * vals[:, None]
    HX = np.zeros((E, D), np.float32)
    np.add.at(HX, cols, msg)
    HX *= de[:, None]
    msg2 = HX[cols] * vals[:, None]
    Xo = np.zeros((N, D), np.float32)
    np.add.at(Xo, rows, msg2)
    Xo *= dv[:, None]
    return Xo @ W.T + b


def kernel(X, h_rows, h_cols, h_vals, DV_inv_sqrt, DE_inv, W, b):
    X = np.asarray(X, dtype=np.float32)
    rows = np.asarray(h_rows).astype(np.int64)
    cols = np.asarray(h_cols).astype(np.int64)
    vals = np.asarray(h_vals, dtype=np.float32)
    dv = np.asarray(DV_inv_sqrt, dtype=np.float32)
    de = np.asarray(DE_inv, dtype=np.float32)
    W = np.asarray(W, dtype=np.float32)
    b = np.asarray(b, dtype=np.float32)

    if not np.all(vals == 1.0):
        return _kernel_np(X, rows, cols, vals, dv, de, W, b).astype(np.float32)

    iota_np = np.broadcast_to(
        np.arange(P, dtype=np.float32).astype(ml_dtypes.bfloat16), (P, P)
    ).copy()
    core_ids = list(range(C))

    # ---- pass 1 ----
    Xn = X * dv[:, None]
    t1 = _hi_lo_table(Xn)
    shard = cols // EPC
    loc_all, idx_all = [], []
    for c in range(C):
        m = np.nonzero(shard == c)[0]
        loc_all.append(cols[m] - c * EPC)
        idx_all.append(rows[m])
    idx1, loc1, na1, nb1, nw1, ws1 = _pack(loc_all, idx_all, EPC, HALF)
    nc1 = _build(na1, nb1, nw1, ws1, pass2=False)
    in1 = [
        {
            "ta": t1[:HALF],
            "tb": t1[HALF:],
            "idx": idx1[c],
            "loc": loc1[c],
            "iota": iota_np,
        }
        for c in range(C)
    ]
    LAST_EXEC_NS.clear()
    LAST_RESULTS.clear()
    res1 = run_bass_kernel_spmd(nc1, in1, core_ids, trace=TRACE)
    LAST_EXEC_NS.append(res1.exec_time_ns)
    LAST_RESULTS.append(res1)
    HX = np.concatenate([res1.results[c]["out"] for c in range(C)], axis=0)

    # ---- pass 2 ----
    HXn = HX.astype(np.float32) * de[:, None]
    t2 = _hi_lo_table(HXn)
    shard2 = rows // NPC
    loc_all, idx_all = [], []
    for c in range(C):
        m = np.nonzero(shard2 == c)[0]
        loc_all.append(rows[m] - c * NPC)
        idx_all.append(cols[m])
    idx2, loc2, na2, nb2, nw2, ws2 = _pack(loc_all, idx_all, NPC, None)
    nc2 = _build(na2, nb2, nw2, ws2, pass2=True)
    wt = _hi_lo_table(np.ascontiguousarray(W.T))
    in2 = [
        {"ta": t2, "idx": idx2[c], "loc": loc2[c], "iota": iota_np, "wt": wt}
        for c in range(C)
    ]
    res2 = run_bass_kernel_spmd(nc2, in2, core_ids, trace=TRACE)
    LAST_EXEC_NS.append(res2.exec_time_ns)
    LAST_RESULTS.append(res2)
    out_t = np.concatenate([res2.results[c]["out"] for c in range(C)], axis=1)
    y = out_t.T  # [N, D] = segsum(no dv) @ W.T
    return np.ascontiguousarray(y * dv[:, None] + b, dtype=np.float32)

